# revision 13
# baseline (speedup 1.0000x reference)
"""GAT symmetry-reduce kernel for Trainium2 (8 NeuronCores, SPMD).

Math (per node i, K=32 neighbors, F=128 features):
    logits[i,k] = leaky_relu(a1_node[i] + 2*a2_node[i] + a2_mail[i,k] + 2*a1_mail[i,k])
    w = softmax_k(logits)
    out[i,f]    = sum_k w[i,k] * ft[i,k,f]

Device strategy (per core: 6272 nodes = 49 tiles of 128):
  - ft rows (node,k) stream in naturally as [128=(4 nodes x 32 k), 128 f]
    chunks and are used as the matmul *stationary* operand.
  - The softmax weights are expanded into a block-diagonal moving operand
    RH[32m+k, q] = w[q,k]*delta(m==q%4) (one broadcast-multiply against a
    constant mask + one PE transpose), so each matmul emits out^T
    [128 f, 4 nodes] straight into PSUM with full partition utilization.
  - Softmax itself runs on DVE/ACT and is hidden under the ft DMA stream.
Output is produced transposed ([F, nodes] per core) and fixed up on host.
"""

import sys

if "/opt/trn_rl_repo" not in sys.path:
    sys.path.insert(0, "/opt/trn_rl_repo")

import numpy as np

import concourse.bacc as bacc
import concourse.bass as bass
import concourse.mybir as mybir
import concourse.tile as tile

N_FULL = 50000
N_CORES = 8
K = 32
F = 128
P = 128                     # nodes per tile / partitions
NT = 49                     # tiles per core
N_SHARD = NT * P            # 6272
N_PAD = N_CORES * N_SHARD   # 50176
NEG_SLOPE = 0.01
OUT_BATCH = 8               # tiles buffered per output DMA

_CACHE = {}
TRACE = False        # test.py sets True to collect an NTFF profile
LAST_RESULT = None   # BassKernelResults from the most recent run


def _build_nc():
    f32 = mybir.dt.float32
    nc = bass.Bass()

    ft = nc.dram_tensor("ft", [N_SHARD * K, F], f32, kind="ExternalInput")
    lpre = nc.dram_tensor("lpre", [P, NT * K], f32, kind="ExternalInput")
    snode = nc.dram_tensor("snode", [P, NT], f32, kind="ExternalInput")
    maskq = nc.dram_tensor("maskq", [P, P], f32, kind="ExternalInput")
    ident = nc.dram_tensor("ident", [P, P], f32, kind="ExternalInput")
    out_t = nc.dram_tensor("out_t", [F, N_SHARD], f32, kind="ExternalOutput")

    # per tile t: [128 part = 32m+k, 32 groups, 128 f]
    ft3 = ft.rearrange("(t g p) f -> t p g f", g=K, p=P)
    lpre3 = lpre.rearrange("p (t k) -> p t k", k=K)

    with tile.TileContext(nc) as tc:
        with (
            tc.tile_pool(name="consts", bufs=1) as consts,
            tc.tile_pool(name="ftp", bufs=3) as ftp,
            tc.tile_pool(name="small", bufs=4) as small,
            tc.tile_pool(name="rhp", bufs=3) as rhp,
            tc.tile_pool(name="psq", bufs=4, space="PSUM") as psq,
            tc.tile_pool(name="pso", bufs=4, space="PSUM") as pso,
            tc.tile_pool(name="oacc", bufs=2) as oacc,
        ):
            identity = consts.tile([P, P], f32)
            nc.sync.dma_start(out=identity, in_=ident[:])
            maskq_sb = consts.tile([P, P], f32)
            nc.sync.dma_start(out=maskq_sb, in_=maskq[:])
            lpre_sb = consts.tile([P, NT, K], f32)
            nc.sync.dma_start(out=lpre_sb, in_=lpre3)
            snode_sb = consts.tile([P, NT], f32)
            nc.sync.dma_start(out=snode_sb, in_=snode[:])

            maskq3 = maskq_sb[:].rearrange("p (m k) -> p m k", m=4)

            out_acc = None
            for t in range(NT):
                ft_tile = ftp.tile([P, K, F], f32, tag="ft")
                nc.sync.dma_start(out=ft_tile, in_=ft3[t])

                # logits = lpre + snode (per-partition scalar), then leaky relu
                logit = small.tile([P, K], f32, tag="logit")
                nc.vector.tensor_scalar_add(
                    out=logit, in0=lpre_sb[:, t, :], scalar1=snode_sb[:, t : t + 1]
                )
                lrelu = small.tile([P, K], f32, tag="lrelu")
                nc.scalar.activation(
                    out=lrelu,
                    in_=logit,
                    func=mybir.ActivationFunctionType.Lrelu,
                    alpha=NEG_SLOPE,
                )
                # softmax over free dim (k)
                mx = small.tile([P, 1], f32, tag="mx")
                nc.vector.reduce_max(out=mx, in_=lrelu, axis=mybir.AxisListType.X)
                negm = small.tile([P, 1], f32, tag="negm")
                nc.vector.tensor_scalar_mul(out=negm, in0=mx, scalar1=-1.0)
                ex = small.tile([P, K], f32, tag="ex")
                ssum = small.tile([P, 1], f32, tag="ssum")
                nc.scalar.activation(
                    out=ex,
                    in_=lrelu,
                    func=mybir.ActivationFunctionType.Exp,
                    bias=negm,
                    scale=1.0,
                    accum_out=ssum,
                )
                rinv = small.tile([P, 1], f32, tag="rinv")
                nc.vector.reciprocal(out=rinv, in_=ssum)
                w = small.tile([P, K], f32, tag="w")
                nc.vector.tensor_scalar_mul(out=w, in0=ex, scalar1=rinv)

                # Q[q, 32m+k] = w[q,k] * maskq  (broadcast w along m via stride-0)
                w_ap = w[:]
                w_b = bass.AP(
                    tensor=w_ap.tensor,
                    offset=w_ap.offset,
                    ap=[w_ap.ap[0], [0, 4], w_ap.ap[1]],
                )
                q_sb = small.tile([P, 4, K], f32, tag="q")
                nc.vector.tensor_tensor(
                    out=q_sb, in0=w_b, in1=maskq3, op=mybir.AluOpType.mult
                )
                # RH = Q^T via PE transpose
                rh_ps = psq.tile([P, P], f32, tag="rhps")
                nc.tensor.transpose(
                    rh_ps, q_sb[:].rearrange("p m k -> p (m k)"), identity[:]
                )
                rh_sb = rhp.tile([P, P], f32, tag="rh")
                nc.vector.tensor_copy(out=rh_sb, in_=rh_ps)

                # 32 matmuls: out^T[f, 4g:4g+4] = ft_chunk^T @ RH[:, 4g:4g+4]
                out_ps = pso.tile([P, P], f32, tag="ops")
                for g in range(K):
                    nc.tensor.matmul(
                        out_ps[:, 4 * g : 4 * g + 4],
                        ft_tile[:, g, :],
                        rh_sb[:, 4 * g : 4 * g + 4],
                        start=True,
                        stop=True,
                    )

                slot = t % OUT_BATCH
                if slot == 0:
                    out_acc = oacc.tile([P, OUT_BATCH * P], f32, tag="oacc")
                nc.vector.tensor_copy(
                    out=out_acc[:, slot * P : (slot + 1) * P], in_=out_ps
                )
                if slot == OUT_BATCH - 1 or t == NT - 1:
                    t0 = t - slot
                    nc.sync.dma_start(
                        out=out_t[:, t0 * P : (t + 1) * P],
                        in_=out_acc[:, : (slot + 1) * P],
                    )
    # TRN2 TS-struct codegen allows one sync wait per compute instruction;
    # split Tile's multi-sem waits into event-semaphore instructions.
    import bass_rust as _bass_rust

    _bass_rust.generate_event_semaphores(nc)
    return nc


def kernel(a1_node, a2_node, a1_mail, a2_mail, ft):
    from concourse.bass_utils import run_bass_kernel_spmd

    a1n = np.asarray(a1_node, np.float32)[:, 0]
    a2n = np.asarray(a2_node, np.float32)[:, 0]
    a1m = np.asarray(a1_mail, np.float32)[:, :, 0]
    a2m = np.asarray(a2_mail, np.float32)[:, :, 0]
    ftf = np.ascontiguousarray(np.asarray(ft, np.float32).reshape(N_FULL * K, F))

    n = a1n.shape[0]
    assert n == N_FULL, n

    lpre = 2.0 * a1m + a2m          # [N, K]
    snode = a1n + 2.0 * a2n         # [N]
    pad = N_PAD - N_FULL
    lpre = np.concatenate([lpre, np.zeros((pad, K), np.float32)], 0)
    snode = np.concatenate([snode, np.zeros((pad,), np.float32)], 0)

    maskq = np.zeros((P, P), np.float32)
    for q in range(P):
        maskq[q, 32 * (q % 4) : 32 * (q % 4) + 32] = 1.0

    in_maps = []
    for c in range(N_CORES):
        lo = c * N_SHARD
        lp = lpre[lo : lo + N_SHARD]        # [6272, K]
        sn = snode[lo : lo + N_SHARD]       # [6272]
        # pre-transpose so SBUF loads are contiguous per partition
        lp_t = np.ascontiguousarray(
            lp.reshape(NT, P, K).transpose(1, 0, 2).reshape(P, NT * K)
        )
        sn_t = np.ascontiguousarray(sn.reshape(NT, P).T)
        row_lo, row_hi = lo * K, (lo + N_SHARD) * K
        if row_hi <= N_FULL * K:
            ft_sh = ftf[row_lo:row_hi]
        else:
            ft_sh = np.concatenate(
                [ftf[row_lo:], np.zeros((row_hi - N_FULL * K, F), np.float32)], 0
            )
        in_maps.append(
            {"ft": ft_sh, "lpre": lp_t, "snode": sn_t, "maskq": maskq,
             "ident": np.eye(P, dtype=np.float32)}
        )

    key = "nc"
    if key not in _CACHE:
        _CACHE[key] = _build_nc()
    res = run_bass_kernel_spmd(
        _CACHE[key], in_maps, list(range(N_CORES)), trace=TRACE
    )
    global LAST_RESULT
    LAST_RESULT = res
    out = np.concatenate([r["out_t"].T for r in res.results], 0)  # [N_PAD, F]
    return np.ascontiguousarray(out[:N_FULL])


if __name__ == "__main__":
    rng = np.random.default_rng(0)
    demo = {
        "a1_node": rng.standard_normal((N_FULL, 1)).astype(np.float32),
        "a2_node": rng.standard_normal((N_FULL, 1)).astype(np.float32),
        "a1_mail": rng.standard_normal((N_FULL, K, 1)).astype(np.float32),
        "a2_mail": rng.standard_normal((N_FULL, K, 1)).astype(np.float32),
        "ft": rng.standard_normal((N_FULL, K, F)).astype(np.float32),
    }
    out = kernel(**demo)
    print("kernel out", out.shape, out.dtype, float(np.abs(out).max()))
